# revision 19
# baseline (speedup 1.0000x reference)
"""Trainium2 Bass kernel for nn_DeepSet_90348932039255 (GNN message passing).

Strategy: partition edges across 8 cores by src-atom range (5000 atoms each).
Each core handles its own edges end-to-end and owns a disjoint slice of the
output atoms, so no collectives are needed. Edges are sorted by src on the
host so the segment-sum becomes a windowed one-hot matmul into an SBUF
accumulator (flushed at a runtime offset per 512-edge group).

Weight fusions (host, fp64):
  gamma = x_src @ W1c.T + x_dst @ W2c.T + ((attr*C) @ W_dp.T) @ Wc1.T
          + C*vbc + ew3*c1 + ews*c2 + w*c3 + bias_total
  W1c = Wg1@W_ai, W2c = Wg2@W_aj, Wc = Wg3@W_dij, Wc1 = Wc[:, :256],
  vbc = Wc1@b_dp, [c1 c2 c3] = Wc[:, 256:259], Wg* = W_gamma column blocks.
  edge_level = sum_g softmax_g(inv) * (W_exp[g] @ gamma + b_exp[g])
The b_exp part is handled by scattering (gates, ev*gates) sums per atom and
folding with b_exp on the host afterwards.
"""

import numpy as np
import ml_dtypes

import concourse.bass as bass
import concourse.bacc as bacc
import concourse.mybir as mybir
import concourse.tile as tile
from concourse import library_config
from concourse.masks import make_identity

F32 = mybir.dt.float32
BF16 = mybir.dt.bfloat16
I16 = mybir.dt.int16
I32 = mybir.dt.int32
AF = mybir.ActivationFunctionType
ALU = mybir.AluOpType
AX = mybir.AxisListType
BF16NP = ml_dtypes.bfloat16

# ---------------- problem constants (hardcoded) ----------------
N_ATOMS = 40000
N_EDGES = 400000
NCORES = 8
OWN = N_ATOMS // NCORES          # 5000
NUM_RBF = 32
OUTER = 5.0
NUM_GATES = 10
EMB = 128

CFG = dict(
    OWN_PAD=5120,
    ACC_N=5248,          # >= 4999+128
    DU_PAD=30720,        # < 32768 (int16)
    G=102,               # 512-edge groups -> EPC = 52224
)

_OFF = np.linspace(0.0, OUTER, NUM_RBF)
COEFF = float(-0.5 / (_OFF[1] - _OFF[0]) ** 2)


def build_program(cfg):
    OWN_PAD, ACC_N, DU_PAD, G = (cfg["OWN_PAD"], cfg["ACC_N"], cfg["DU_PAD"],
                                 cfg["G"])
    nc = bacc.Bacc()
    P = {}

    def inp(name, shape, dt):
        P[name] = nc.declare_dram_parameter(name, list(shape), dt, isOutput=False)

    inp("xs_tab", [OWN_PAD, 128], BF16)
    inp("xd_tab", [DU_PAD, 128], BF16)
    inp("ps_tab", [OWN_PAD, 64], F32)
    inp("pd_tab", [DU_PAD, 64], F32)
    inp("src_w", [G, 128, 32], I16)
    inp("dst_w", [G, 128, 32], I16)
    inp("srcrel", [G, 128, 4], F32)
    inp("a0s", [1, G], I32)
    inp("wdpT", [64, 2, 128], BF16)        # [k=attr, mh, m]
    inp("wc1T", [128, 2, 2, 128], BF16)    # [k_lo, kh, mh, m]
    inp("w1cT", [128, 2, 128], BF16)       # [k, mh, m]
    inp("w2cT", [128, 2, 128], BF16)
    inp("r1", [16, 2, 128], BF16)          # [feat, mh, m]
    inp("wexpT", [128, 10, 2, 128], BF16)  # [k_lo, g, kh, m]
    inp("tmat", [128, 4, 16], F32)
    inp("offmat", [128, 4, 32], F32)
    inp("iota", [128, 128], F32)
    inp("bias_g", [128, 2], F32)
    P["out_w"] = nc.declare_dram_parameter("out_w", [G, 128, 512], F32,
                                           isOutput=True)
    P["outg_w"] = nc.declare_dram_parameter("outg_w", [G, 64, 128], F32,
                                            isOutput=True)

    import contextlib
    with tile.TileContext(nc) as tc, contextlib.ExitStack() as ctx:
        nc.gpsimd.load_library(library_config.mlp)
        cpool = ctx.enter_context(tc.tile_pool(name="const", bufs=1))
        apool = ctx.enter_context(tc.tile_pool(name="accp", bufs=1))
        gpool = ctx.enter_context(tc.tile_pool(name="gath", bufs=3))
        wpool = ctx.enter_context(tc.tile_pool(name="work", bufs=2))
        bpool = ctx.enter_context(tc.tile_pool(name="big", bufs=2))
        scpool = ctx.enter_context(tc.tile_pool(name="scale", bufs=5))
        ps_mm = ctx.enter_context(tc.tile_pool(name="psmm", bufs=3, space="PSUM"))
        ps_sc = ctx.enter_context(tc.tile_pool(name="pssc", bufs=2, space="PSUM"))
        ps_g5 = ctx.enter_context(tc.tile_pool(name="psg5", bufs=1, space="PSUM"))
        ps_t = ctx.enter_context(tc.tile_pool(name="pst", bufs=2, space="PSUM"))

        def cload(name, shape, dt):
            t = cpool.tile(shape, dt, tag=name, name=name + "_sb")
            nc.sync.dma_start(t[:], P[name][:])
            return t

        wdp_sb = cload("wdpT", [64, 2, 128], BF16)
        wc1_sb = cload("wc1T", [128, 2, 2, 128], BF16)
        w1c_sb = cload("w1cT", [128, 2, 128], BF16)
        w2c_sb = cload("w2cT", [128, 2, 128], BF16)
        r1_sb = cload("r1", [16, 2, 128], BF16)
        wexp_sb = cload("wexpT", [128, 10, 2, 128], BF16)
        tmat_sb = cload("tmat", [128, 4, 16], F32)
        offm_sb = cload("offmat", [128, 4, 32], F32)
        iota_sb = cload("iota", [128, 128], F32)
        bias_sb = cload("bias_g", [128, 2], F32)
        a0_sb = cload("a0s", [1, G], I32)
        eye_b = cpool.tile([128, 128], BF16)
        make_identity(nc, eye_b[:])
        eye_f = cpool.tile([128, 128], F32)
        make_identity(nc, eye_f[:])
        halfpi = cpool.tile([128, 1], F32)
        nc.vector.memset(halfpi[:], float(np.pi / 2))

        v512 = nc.gpsimd.alloc_register("n512")
        nc.gpsimd.reg_mov(v512, 512)

        for g in range(G):
            # ---------------- gathers ----------------
            sidx = gpool.tile([128, 32], I16, tag="sidx")
            nc.sync.dma_start(sidx[:], P["src_w"][g])
            didx = gpool.tile([128, 32], I16, tag="didx")
            nc.sync.dma_start(didx[:], P["dst_w"][g])
            srf = gpool.tile([128, 4], F32, tag="srf")
            nc.sync.dma_start(srf[:], P["srcrel"][g])

            xs = gpool.tile([128, 512], BF16, tag="xs")
            nc.gpsimd.dma_gather(
                out_ap=xs[:].rearrange("p (c n) -> p c n", c=1),
                in_ap=P["xs_tab"][:], idxs_ap=sidx[:],
                num_idxs=512, num_idxs_reg=v512, elem_size=128, transpose=True)
            xd = gpool.tile([128, 512], BF16, tag="xd")
            nc.gpsimd.dma_gather(
                out_ap=xd[:].rearrange("p (c n) -> p c n", c=1),
                in_ap=P["xd_tab"][:], idxs_ap=didx[:],
                num_idxs=512, num_idxs_reg=v512, elem_size=128, transpose=True)
            pos_s = gpool.tile([128, 4, 64], F32, tag="pos_s")
            nc.gpsimd.dma_gather(
                out_ap=pos_s[:], in_ap=P["ps_tab"][:], idxs_ap=sidx[:],
                num_idxs=512, num_idxs_reg=v512, elem_size=64, transpose=False)
            pos_d = gpool.tile([128, 4, 64], F32, tag="pos_d")
            nc.gpsimd.dma_gather(
                out_ap=pos_d[:], in_ap=P["pd_tab"][:], idxs_ap=didx[:],
                num_idxs=512, num_idxs_reg=v512, elem_size=64, transpose=False)

            # ------------- distance path (chunk [128, 4, *]) -------------
            d = wpool.tile([128, 4, 3], F32, tag="d")
            nc.vector.tensor_tensor(out=d[:], in0=pos_s[:, :, 0:3],
                                    in1=pos_d[:, :, 0:3], op=ALU.subtract)
            sq = wpool.tile([128, 4, 3], F32, tag="sq")
            nc.vector.tensor_tensor(out=sq[:], in0=d[:], in1=d[:], op=ALU.mult)
            s2 = wpool.tile([128, 4, 1], F32, tag="s2")
            nc.vector.tensor_reduce(out=s2[:], in_=sq[:], axis=AX.X, op=ALU.add)
            wt = wpool.tile([128, 4, 1], F32, tag="wt")
            w_ap = wt[:]
            nc.scalar.activation(w_ap, s2[:], AF.Sqrt)
            invw = wpool.tile([128, 4, 1], F32, tag="invw")
            nc.vector.reciprocal(invw[:], w_ap)
            iwt = wpool.tile([128, 4, 1], F32, tag="iwt")
            nc.vector.tensor_tensor(out=iwt[:], in0=w_ap, in1=invw[:], op=ALU.mult)
            nc.vector.tensor_scalar(iwt[:], iwt[:], 2.0, -1.0, ALU.subtract,
                                    ALU.mult)
            nc.vector.tensor_tensor(out=invw[:], in0=invw[:], in1=iwt[:],
                                    op=ALU.mult)
            ev = wpool.tile([128, 4, 3], F32, tag="ev")
            nc.vector.tensor_tensor(out=ev[:], in0=d[:],
                                    in1=invw[:].to_broadcast([128, 4, 3]),
                                    op=ALU.mult)
            ew3 = wpool.tile([128, 4, 1], F32, tag="ew3")
            nc.vector.tensor_tensor(out=ew3[:], in0=w_ap, in1=s2[:], op=ALU.mult)
            # cutoff C = 0.5*(cos(pi*w/5)+1) * (w < 5)
            wcl = wpool.tile([128, 4, 1], F32, tag="wcl")
            nc.vector.tensor_scalar(wcl[:], w_ap, 5.0, None, ALU.min)
            sinv = wpool.tile([128, 4, 1], F32, tag="sinv")
            nc.scalar.activation(sinv[:], wcl[:], AF.Sin,
                                 bias=halfpi[:, 0:1], scale=float(-np.pi / OUTER))
            mlt = wpool.tile([128, 4, 1], F32, tag="mlt")
            nc.vector.tensor_scalar(mlt[:], w_ap, 5.0, None, ALU.is_lt)
            c0 = wpool.tile([128, 4, 1], F32, tag="c0")
            nc.vector.tensor_scalar(c0[:], sinv[:], 0.5, 0.5, ALU.mult, ALU.add)
            cC = wpool.tile([128, 4, 1], F32, tag="cC")
            nc.vector.tensor_tensor(out=cC[:], in0=c0[:], in1=mlt[:], op=ALU.mult)

            # bundle (bf16): [C, ew3, ews, w, 0...]
            bundle = wpool.tile([128, 4, 16], BF16, tag="bundle")
            nc.gpsimd.memset(bundle[:], 0.0)
            nc.vector.tensor_copy(bundle[:, :, 0:1], cC[:])
            nc.vector.tensor_copy(bundle[:, :, 1:2], ew3[:])
            nc.scalar.activation(bundle[:, :, 2:3], w_ap, AF.Sqrt)   # ews
            nc.vector.tensor_copy(bundle[:, :, 3:4], w_ap)

            # RBF: attr = exp(coeff*(off - x)^2), x in {w, w^3}; then * C
            q = wpool.tile([128, 4, 64], F32, tag="q")
            nc.vector.tensor_tensor(out=q[:, :, 0:32], in0=offm_sb[:],
                                    in1=w_ap.to_broadcast([128, 4, 32]),
                                    op=ALU.subtract)
            nc.vector.tensor_tensor(out=q[:, :, 32:64], in0=offm_sb[:],
                                    in1=ew3[:].to_broadcast([128, 4, 32]),
                                    op=ALU.subtract)
            q2 = wpool.tile([128, 4, 64], F32, tag="q2")
            nc.scalar.activation(q2[:], q[:], AF.Square)
            attr_f = wpool.tile([128, 4, 64], F32, tag="attr_f")
            nc.scalar.activation(attr_f[:], q2[:], AF.Exp, scale=COEFF)
            attr_b = wpool.tile([128, 4, 64], BF16, tag="attr_b")
            nc.vector.tensor_tensor(out=attr_b[:], in0=attr_f[:],
                                    in1=cC[:].to_broadcast([128, 4, 64]),
                                    op=ALU.mult)

            # ------------- gates (e-partition chain) -------------
            tdiff = wpool.tile([128, 4, 16], F32, tag="tdiff")
            nc.vector.tensor_tensor(out=tdiff[:], in0=tmat_sb[:],
                                    in1=w_ap.to_broadcast([128, 4, 16]),
                                    op=ALU.subtract)
            ntd = wpool.tile([128, 4, 16], F32, tag="ntd")
            nc.vector.tensor_scalar(ntd[:], tdiff[:], -1.0, None, ALU.mult)
            y = wpool.tile([128, 4, 16], F32, tag="y")
            nc.vector.tensor_tensor(out=y[:], in0=tdiff[:], in1=ntd[:],
                                    op=ALU.max)
            nc.vector.tensor_scalar(y[:], y[:], 1e-8, None, ALU.max)
            inv0 = wpool.tile([128, 4, 16], F32, tag="inv0")
            nc.vector.reciprocal(inv0[:], y[:])
            nt1 = wpool.tile([128, 4, 16], F32, tag="nt1")
            nc.vector.tensor_tensor(out=nt1[:], in0=y[:], in1=inv0[:], op=ALU.mult)
            nc.vector.tensor_scalar(nt1[:], nt1[:], 2.0, -1.0, ALU.subtract,
                                    ALU.mult)
            inv2 = wpool.tile([128, 4, 16], F32, tag="inv2")
            nc.vector.tensor_tensor(out=inv2[:], in0=inv0[:], in1=nt1[:],
                                    op=ALU.mult)
            mne = wpool.tile([128, 4, 1], F32, tag="mne")
            nc.vector.tensor_reduce(out=mne[:], in_=inv2[:, :, 0:10], axis=AX.X,
                                    op=ALU.max, negate=True)
            hm = wpool.tile([128, 4, 10], F32, tag="hm")
            nc.vector.tensor_tensor(out=hm[:], in0=inv2[:, :, 0:10],
                                    in1=mne[:].to_broadcast([128, 4, 10]),
                                    op=ALU.add)
            gexp = wpool.tile([128, 4, 16], F32, tag="gexp")
            nc.gpsimd.memset(gexp[:], 0.0)
            nc.scalar.activation(gexp[:, :, 0:10], hm[:], AF.Exp)
            s_e = wpool.tile([128, 4, 1], F32, tag="s_e")
            nc.vector.tensor_reduce(out=s_e[:], in_=gexp[:, :, 0:10], axis=AX.X,
                                    op=ALU.add)
            rs_e = wpool.tile([128, 4, 1], F32, tag="rs_e")
            nc.vector.reciprocal(rs_e[:], s_e[:])
            rt1 = wpool.tile([128, 4, 1], F32, tag="rt1")
            nc.vector.tensor_tensor(out=rt1[:], in0=s_e[:], in1=rs_e[:],
                                    op=ALU.mult)
            nc.vector.tensor_scalar(rt1[:], rt1[:], 2.0, -1.0, ALU.subtract,
                                    ALU.mult)
            nc.vector.tensor_tensor(out=rs_e[:], in0=rs_e[:], in1=rt1[:],
                                    op=ALU.mult)
            # normalized gates (bf16) + ev-scaled copies for the bias scatter
            gb = wpool.tile([128, 4, 64], BF16, tag="gb")
            nc.gpsimd.memset(gb[:], 0.0)
            nc.vector.tensor_tensor(out=gb[:, :, 0:10], in0=gexp[:, :, 0:10],
                                    in1=rs_e[:].to_broadcast([128, 4, 10]),
                                    op=ALU.mult)
            for i in range(3):
                nc.vector.tensor_tensor(
                    out=gb[:, :, 16 * (i + 1):16 * (i + 1) + 10],
                    in0=gb[:, :, 0:10],
                    in1=ev[:, :, i:i + 1].to_broadcast([128, 4, 10]),
                    op=ALU.mult)
            # rs*ev slot scales (f32)
            rsev = wpool.tile([128, 4, 3], F32, tag="rsev")
            nc.vector.tensor_tensor(out=rsev[:], in0=ev[:],
                                    in1=rs_e[:].to_broadcast([128, 4, 3]),
                                    op=ALU.mult)

            # ------------- per-subtile transposes -------------
            attrT = bpool.tile([64, 512], BF16, tag="attrT")
            bunT = bpool.tile([16, 512], BF16, tag="bunT")
            gexpT = bpool.tile([16, 512], BF16, tag="gexpT")
            for s in range(4):
                pt1 = ps_t.tile([64, 128], BF16, tag="pst", space="PSUM")
                nc.tensor.transpose(out=pt1[:], in_=attr_b[:, s], identity=eye_b[:])
                nc.scalar.copy(attrT[:, 128 * s:128 * (s + 1)], pt1[:])
                pt2 = ps_t.tile([16, 128], BF16, tag="pst", space="PSUM")
                nc.tensor.transpose(out=pt2[:], in_=bundle[:, s], identity=eye_b[:])
                nc.vector.tensor_copy(bunT[:, 128 * s:128 * (s + 1)], pt2[:])
                pt3 = ps_t.tile([16, 128], F32, tag="pst", space="PSUM")
                nc.tensor.transpose(out=pt3[:], in_=gexp[:, s], identity=eye_f[:])
                nc.vector.tensor_copy(gexpT[:, 128 * s:128 * (s + 1)], pt3[:])

            gexF = bpool.tile([1, 10, 512], BF16, tag="gexF")
            nc.sync.dma_start(gexF[:], gexpT[0:10, :])

            # ------------- dp + gamma matmuls -------------
            dp_sb = bpool.tile([128, 2, 512], BF16, tag="dp_sb")
            for m in range(2):
                pdp = ps_mm.tile([128, 512], F32, tag="psmm", space="PSUM")
                nc.tensor.matmul(pdp[:], lhsT=wdp_sb[:, m], rhs=attrT[:],
                                 start=True, stop=True)
                nc.scalar.copy(dp_sb[:, m], pdp[:])
            gam_sb = bpool.tile([128, 2, 512], BF16, tag="gam_sb")
            for m in range(2):
                pg = ps_mm.tile([128, 512], F32, tag="psmm", space="PSUM")
                nc.tensor.matmul(pg[:], lhsT=wc1_sb[:, 0, m], rhs=dp_sb[:, 0],
                                 start=True, stop=False)
                nc.tensor.matmul(pg[:], lhsT=wc1_sb[:, 1, m], rhs=dp_sb[:, 1],
                                 start=False, stop=False)
                nc.tensor.matmul(pg[:], lhsT=w1c_sb[:, m], rhs=xs[:],
                                 start=False, stop=False)
                nc.tensor.matmul(pg[:], lhsT=w2c_sb[:, m], rhs=xd[:],
                                 start=False, stop=False)
                nc.tensor.matmul(pg[:], lhsT=r1_sb[:, m], rhs=bunT[:],
                                 start=False, stop=True)
                nc.scalar.activation(gam_sb[:, m], pg[:], AF.Identity,
                                     bias=bias_sb[:, m:m + 1])

            # ------------- experts with gate prescale -------------
            pel = ps_mm.tile([128, 512], F32, tag="psmm", space="PSUM")
            for g10 in range(NUM_GATES):
                gbc = scpool.tile([128, 512], BF16, tag="gbc")
                nc.gpsimd.partition_broadcast(gbc[:], gexF[:, g10])
                gs0 = scpool.tile([128, 512], BF16, tag="gs0")
                nc.vector.tensor_tensor(out=gs0[:], in0=gam_sb[:, 0], in1=gbc[:],
                                        op=ALU.mult)
                gs1 = scpool.tile([128, 512], BF16, tag="gs1")
                nc.vector.tensor_tensor(out=gs1[:], in0=gam_sb[:, 1], in1=gbc[:],
                                        op=ALU.mult)
                nc.tensor.matmul(pel[:], lhsT=wexp_sb[:, g10, 0], rhs=gs0[:],
                                 start=(g10 == 0), stop=False)
                nc.tensor.matmul(pel[:], lhsT=wexp_sb[:, g10, 1], rhs=gs1[:],
                                 start=False, stop=(g10 == NUM_GATES - 1))
            el_b = bpool.tile([128, 512], BF16, tag="el_b")
            nc.scalar.copy(el_b[:], pel[:])

            # ------------- edge_level transpose + slots + scatter -------------
            psc = ps_sc.tile([128, 4, 128], F32, tag="pssc", space="PSUM")
            psc5 = ps_g5.tile([64, 128], F32, tag="psg5", space="PSUM")
            elTs, ohs = [], []
            for s in range(4):
                ptl = ps_t.tile([128, 128], BF16, tag="pst", space="PSUM")
                nc.tensor.transpose(out=ptl[:], in_=el_b[:, 128 * s:128 * (s + 1)],
                                    identity=eye_b[:])
                elT = scpool.tile([128, 4, 128], BF16, tag="elT", name=f"elT{s}")
                nc.scalar.activation(elT[:, 0], ptl[:], AF.Copy,
                                     scale=rs_e[:, s])
                for i in range(3):
                    nc.scalar.activation(elT[:, i + 1], ptl[:], AF.Copy,
                                         scale=rsev[:, s, i:i + 1])
                oh = scpool.tile([128, 128], BF16, tag="oh", name=f"oh{s}")
                nc.vector.tensor_tensor(out=oh[:],
                                        in0=srf[:, s:s + 1].to_broadcast([128, 128]),
                                        in1=iota_sb[:], op=ALU.is_equal)
                elTs.append(elT)
                ohs.append(oh)
                nc.tensor.matmul(psc5[:], lhsT=gb[:, s], rhs=oh[:],
                                 start=(s == 0), stop=(s == 3))
            for t in range(4):
                for s in range(4):
                    nc.tensor.matmul(psc[:, t], lhsT=elTs[s][:, t], rhs=ohs[s],
                                     start=(s == 0), stop=(s == 3))

            # ------------- window flush to DRAM -------------
            fl = bpool.tile([128, 4, 128], F32, tag="fl")
            nc.scalar.copy(fl[:], psc[:])
            nc.sync.dma_start(P["out_w"][g],
                              fl[:].rearrange("p t a -> p (t a)"))
            fl5 = bpool.tile([64, 128], F32, tag="fl5")
            nc.vector.tensor_copy(fl5[:], psc5[:])
            nc.sync.dma_start(P["outg_w"][g], fl5[:])


    nc.finalize()
    return nc, P


# ====================== host-side preparation ======================

def prep_core(src, dst, xz_bf16, pos, cfg):
    """Build the per-core input map. src/dst: this core's edges (global ids,
    sorted by src). xz_bf16: [N_ATOMS,128] bf16 node features. pos: [N,3]."""
    OWN_PAD, ACC_N, DU_PAD, G = (cfg["OWN_PAD"], cfg["ACC_N"], cfg["DU_PAD"],
                                 cfg["G"])
    EPC = 512 * G
    core = int(src[0]) // OWN if len(src) else 0
    base = core * OWN
    n = len(src)
    assert n <= EPC, (n, EPC)

    src_local = (src - base).astype(np.int64)
    du = np.unique(dst)
    assert len(du) <= DU_PAD - 1, len(du)
    dstc = np.searchsorted(du, dst).astype(np.int64)

    sl_p = np.full(EPC, OWN_PAD - 1, np.int64)
    sl_p[:n] = src_local
    dc_p = np.full(EPC, DU_PAD - 1, np.int64)
    dc_p[:n] = dstc

    A0 = np.zeros(G, np.int32)
    srcrel = np.full(EPC, -1000.0, np.float32)
    for g in range(G):
        lo = g * 512
        hi = min(lo + 512, n)
        if lo < n:
            a0 = int(sl_p[lo])
            A0[g] = a0
            srcrel[lo:hi] = (sl_p[lo:hi] - a0).astype(np.float32)
            assert sl_p[hi - 1] - a0 <= 127, "window overflow"

    def wrap(idx):
        w = idx.astype(np.int16).reshape(G, 32, 16)
        w = np.transpose(w, (0, 2, 1))          # [G, 16, 32]
        return np.ascontiguousarray(np.tile(w, (1, 8, 1)))

    src_w = wrap(sl_p)
    dst_w = wrap(dc_p)
    srcrel_T = np.ascontiguousarray(
        np.transpose(srcrel.reshape(G, 4, 128), (0, 2, 1)))

    xs_tab = np.zeros((OWN_PAD, 128), BF16NP)
    xs_tab[:OWN] = xz_bf16[base:base + OWN]
    xd_tab = np.zeros((DU_PAD, 128), BF16NP)
    xd_tab[:len(du)] = xz_bf16[du]

    ps_tab = np.zeros((OWN_PAD, 64), np.float32)
    ps_tab[:OWN, 0:3] = pos[base:base + OWN]
    pd_tab = np.ones((DU_PAD, 64), np.float32)
    pd_tab[:len(du), 0:3] = pos[du]
    pd_tab[len(du):, 0:3] = 1.0  # pad rows: finite, != src pad pos

    return dict(xs_tab=xs_tab, xd_tab=xd_tab, ps_tab=ps_tab, pd_tab=pd_tab,
                src_w=src_w, dst_w=dst_w, srcrel=srcrel_T,
                a0s=A0.reshape(1, G)), du, n


def prep_weights(inputs, cfg):
    """Fused weight tensors shared by all cores (host math in float64)."""
    W_gamma = np.asarray(inputs["W_gamma"], np.float64)
    Wg1 = W_gamma[:, 0:128]
    Wg2 = W_gamma[:, 128:384]
    Wg3 = W_gamma[:, 384:512]
    W_ai = np.asarray(inputs["W_ai"], np.float64)
    W_aj = np.asarray(inputs["W_aj"], np.float64)
    W_dij = np.asarray(inputs["W_dij"], np.float64)
    W_dp = np.asarray(inputs["W_dp"], np.float64)
    b_dp = np.asarray(inputs["b_dp"], np.float64)
    b_ai = np.asarray(inputs["b_ai"], np.float64)
    b_aj = np.asarray(inputs["b_aj"], np.float64)
    b_dij = np.asarray(inputs["b_dij"], np.float64)
    b_gamma = np.asarray(inputs["b_gamma"], np.float64)
    W_exp = np.asarray(inputs["W_exp"], np.float64)   # [10, 128, 256]
    t_params = np.asarray(inputs["t_params"], np.float64)

    W1c = Wg1 @ W_ai                    # [256, 128]
    W2c = Wg2 @ W_aj                    # [256, 128]
    Wc = Wg3 @ W_dij                    # [256, 259]
    Wc1 = Wc[:, 0:256]
    vbc = Wc1 @ b_dp                    # [256]
    bias_total = b_gamma + Wg1 @ b_ai + Wg2 @ b_aj + Wg3 @ b_dij

    def b16(x):
        return np.ascontiguousarray(x.astype(np.float32)).astype(BF16NP)

    # wdpT [k=64 attr, mh, m]
    wdpT = b16(W_dp.T.reshape(64, 2, 128))
    # wc1T [k_lo, kh, mh, m] from Wc1.T [k=256, m=256]
    wc1T = b16(np.transpose(Wc1.T.reshape(2, 128, 2, 128), (1, 0, 2, 3)))
    w1cT = b16(W1c.T.reshape(128, 2, 128))
    w2cT = b16(W2c.T.reshape(128, 2, 128))
    r1 = np.zeros((16, 2, 128), BF16NP)
    r1[0] = b16(vbc.reshape(2, 128))
    r1[1] = b16(Wc[:, 256].reshape(2, 128))
    r1[2] = b16(Wc[:, 257].reshape(2, 128))
    r1[3] = b16(Wc[:, 258].reshape(2, 128))
    # wexpT [k_lo, g, kh, m] from W_exp[g].T [k=256, m=128]
    wexpT = b16(np.transpose(
        np.transpose(W_exp, (0, 2, 1)).reshape(10, 2, 128, 128), (2, 0, 1, 3)))

    t_pad = np.full(16, 1e9, np.float32)
    t_pad[:NUM_GATES] = t_params.astype(np.float32)
    tmat = np.ascontiguousarray(
        np.broadcast_to(t_pad, (128, 4, 16)).astype(np.float32))
    offm = np.ascontiguousarray(
        np.broadcast_to(_OFF.astype(np.float32), (128, 4, 32)))
    iota = np.ascontiguousarray(
        np.broadcast_to(np.arange(128, dtype=np.float32), (128, 128)))
    bias_g = np.ascontiguousarray(
        bias_total.astype(np.float32).reshape(2, 128).T)   # [128, 2]

    return dict(wdpT=wdpT, wc1T=wc1T, w1cT=w1cT, w2cT=w2cT, r1=r1,
                wexpT=wexpT, tmat=tmat, offmat=offm, iota=iota, bias_g=bias_g)


_PROG_CACHE = {}
LAST_EXEC_NS = None
LAST_RESULTS = None


def kernel(**inputs):
    cfg = CFG
    z = np.asarray(inputs["z"])
    pos = np.asarray(inputs["pos"], np.float32)
    batch = np.asarray(inputs["batch"])
    edge_index = np.asarray(inputs["edge_index"])
    emb_table = np.asarray(inputs["emb_table"], np.float32)
    b_exp = np.asarray(inputs["b_exp"], np.float32)     # [10, 128]

    src = edge_index[0].astype(np.int64)
    dst = edge_index[1].astype(np.int64)
    xz = emb_table[z].astype(BF16NP)                    # [N, 128] bf16

    wmaps = prep_weights(inputs, cfg)

    in_maps = []
    metas = []
    core_id = src // OWN
    for c in range(NCORES):
        m = core_id == c
        es, ed = src[m], dst[m]
        order = np.argsort(es, kind="stable")
        es, ed = es[order], ed[order]
        core_in, du, n = prep_core(es, ed, xz, pos, cfg)
        core_in.update(wmaps)
        in_maps.append(core_in)
        metas.append(du)

    key = "full"
    if key not in _PROG_CACHE:
        _PROG_CACHE[key] = build_program(cfg)
    nc, _ = _PROG_CACHE[key]

    import os
    from concourse.bass_utils import run_bass_kernel_spmd
    trace = bool(int(os.environ.get("KERNEL_TRACE", "0")))
    try:
        res = run_bass_kernel_spmd(nc, in_maps, list(range(NCORES)), trace=trace)
    except ModuleNotFoundError:
        res = run_bass_kernel_spmd(nc, in_maps, list(range(NCORES)))
    global LAST_EXEC_NS, LAST_RESULTS
    LAST_EXEC_NS = res.exec_time_ns
    LAST_RESULTS = res

    atom_x = np.zeros((N_ATOMS, 128), np.float32)
    vec = np.zeros((N_ATOMS, 3, 128), np.float32)
    G = cfg["G"]
    ACC_N = cfg["ACC_N"]
    for c in range(NCORES):
        out_w = res.results[c]["out_w"]        # [G, 128, 512]
        outg_w = res.results[c]["outg_w"]      # [G, 64, 128]
        A0 = np.asarray(in_maps[c]["a0s"]).reshape(G)
        out_t = np.zeros((512, ACC_N), np.float64)
        accg = np.zeros((64, ACC_N), np.float64)
        for g in range(G):
            a0 = int(A0[g])
            out_t[:, a0:a0 + 128] += out_w[g].reshape(128, 4, 128).transpose(
                1, 0, 2).reshape(512, 128)
            accg[:, a0:a0 + 128] += outg_w[g]
        full = out_t[:, :OWN].T                # [OWN, 512]
        gsum = accg[0:10, :OWN].T              # [OWN, 10]
        gev = accg[16:64, :OWN].reshape(3, 16, OWN)[:, 0:10]  # [3, 10, OWN]
        ax = full[:, 0:128] + gsum @ b_exp
        vv = full[:, 128:512].reshape(OWN, 3, 128)
        vv = vv + np.einsum("ign,gc->nic", gev, b_exp)
        atom_x[c * OWN:(c + 1) * OWN] = ax.astype(np.float32)
        vec[c * OWN:(c + 1) * OWN] = vv.astype(np.float32)

    return (atom_x, vec, np.asarray(inputs["z"]), np.asarray(inputs["pos"]),
            np.asarray(inputs["batch"]))


# revision 20
# speedup vs baseline: 1.4319x; 1.4319x over previous
"""Trainium2 Bass kernel for nn_DeepSet_90348932039255 (GNN message passing).

Strategy: partition edges across 8 cores by src-atom range (5000 atoms each).
Each core handles its own edges end-to-end and owns a disjoint slice of the
output atoms, so no collectives are needed. Edges are sorted by src on the
host so the segment-sum becomes a windowed one-hot matmul into an SBUF
accumulator (flushed at a runtime offset per 512-edge group).

Weight fusions (host, fp64):
  gamma = x_src @ W1c.T + x_dst @ W2c.T + ((attr*C) @ W_dp.T) @ Wc1.T
          + C*vbc + ew3*c1 + ews*c2 + w*c3 + bias_total
  W1c = Wg1@W_ai, W2c = Wg2@W_aj, Wc = Wg3@W_dij, Wc1 = Wc[:, :256],
  vbc = Wc1@b_dp, [c1 c2 c3] = Wc[:, 256:259], Wg* = W_gamma column blocks.
  edge_level = sum_g softmax_g(inv) * (W_exp[g] @ gamma + b_exp[g])
The b_exp part is handled by scattering (gates, ev*gates) sums per atom and
folding with b_exp on the host afterwards.
"""

import numpy as np
import ml_dtypes

import concourse.bass as bass
import concourse.bacc as bacc
import concourse.mybir as mybir
import concourse.tile as tile
from concourse import library_config
from concourse.masks import make_identity

F32 = mybir.dt.float32
BF16 = mybir.dt.bfloat16
I16 = mybir.dt.int16
I32 = mybir.dt.int32
AF = mybir.ActivationFunctionType
ALU = mybir.AluOpType
AX = mybir.AxisListType
BF16NP = ml_dtypes.bfloat16

# ---------------- problem constants (hardcoded) ----------------
N_ATOMS = 40000
N_EDGES = 400000
NCORES = 8
OWN = N_ATOMS // NCORES          # 5000
NUM_RBF = 32
OUTER = 5.0
NUM_GATES = 10
EMB = 128

CFG = dict(
    OWN_PAD=5120,
    ACC_N=5248,          # >= 4999+128
    DU_PAD=30720,        # < 32768 (int16)
    G=102,               # 512-edge groups -> EPC = 52224
)

_OFF = np.linspace(0.0, OUTER, NUM_RBF)
COEFF = float(-0.5 / (_OFF[1] - _OFF[0]) ** 2)


def build_program(cfg):
    OWN_PAD, ACC_N, DU_PAD, G = (cfg["OWN_PAD"], cfg["ACC_N"], cfg["DU_PAD"],
                                 cfg["G"])
    nc = bacc.Bacc()
    P = {}

    def inp(name, shape, dt):
        P[name] = nc.declare_dram_parameter(name, list(shape), dt, isOutput=False)

    inp("xs_tab", [OWN_PAD, 128], BF16)
    inp("xd_tab", [DU_PAD, 128], BF16)
    inp("ps_tab", [OWN_PAD, 64], F32)
    inp("pd_tab", [DU_PAD, 64], F32)
    inp("src_w", [G, 128, 32], I16)
    inp("dst_w", [G, 128, 32], I16)
    inp("srcrel", [G, 128, 4], F32)
    inp("a0s", [1, G], I32)
    inp("wdpT", [64, 2, 128], BF16)        # [k=attr, mh, m]
    inp("wc1T", [128, 2, 2, 128], BF16)    # [k_lo, kh, mh, m]
    inp("w1cT", [128, 2, 128], BF16)       # [k, mh, m]
    inp("w2cT", [128, 2, 128], BF16)
    inp("r1", [16, 2, 128], BF16)          # [feat, mh, m]
    inp("wexpT", [128, 10, 2, 128], BF16)  # [k_lo, g, kh, m]
    inp("tmat", [128, 4, 16], F32)
    inp("offmat", [128, 4, 32], F32)
    inp("iota", [128, 128], F32)
    inp("bias_g", [128, 2], F32)
    P["out_w"] = nc.declare_dram_parameter("out_w", [G, 128, 512], F32,
                                           isOutput=True)
    P["outg_w"] = nc.declare_dram_parameter("outg_w", [G, 64, 128], F32,
                                            isOutput=True)

    import contextlib
    with tile.TileContext(nc) as tc, contextlib.ExitStack() as ctx:
        nc.gpsimd.load_library(library_config.mlp)
        cpool = ctx.enter_context(tc.tile_pool(name="const", bufs=1))
        apool = ctx.enter_context(tc.tile_pool(name="accp", bufs=1))
        gpool = ctx.enter_context(tc.tile_pool(name="gath", bufs=5))
        wpool = ctx.enter_context(tc.tile_pool(name="work", bufs=3))
        bpool = ctx.enter_context(tc.tile_pool(name="big", bufs=3))
        scpool = ctx.enter_context(tc.tile_pool(name="scale", bufs=6))
        ps_mm = ctx.enter_context(tc.tile_pool(name="psmm", bufs=3, space="PSUM"))
        ps_sc = ctx.enter_context(tc.tile_pool(name="pssc", bufs=2, space="PSUM"))
        ps_g5 = ctx.enter_context(tc.tile_pool(name="psg5", bufs=1, space="PSUM"))
        ps_t = ctx.enter_context(tc.tile_pool(name="pst", bufs=2, space="PSUM"))

        def cload(name, shape, dt):
            t = cpool.tile(shape, dt, tag=name, name=name + "_sb")
            nc.sync.dma_start(t[:], P[name][:])
            return t

        wdp_sb = cload("wdpT", [64, 2, 128], BF16)
        wc1_sb = cload("wc1T", [128, 2, 2, 128], BF16)
        w1c_sb = cload("w1cT", [128, 2, 128], BF16)
        w2c_sb = cload("w2cT", [128, 2, 128], BF16)
        r1_sb = cload("r1", [16, 2, 128], BF16)
        wexp_sb = cload("wexpT", [128, 10, 2, 128], BF16)
        tmat_sb = cload("tmat", [128, 4, 16], F32)
        offm_sb = cload("offmat", [128, 4, 32], F32)
        iota_sb = cload("iota", [128, 128], F32)
        bias_sb = cload("bias_g", [128, 2], F32)
        a0_sb = cload("a0s", [1, G], I32)
        eye_b = cpool.tile([128, 128], BF16)
        make_identity(nc, eye_b[:])
        eye_f = cpool.tile([128, 128], F32)
        make_identity(nc, eye_f[:])
        halfpi = cpool.tile([128, 1], F32)
        nc.vector.memset(halfpi[:], float(np.pi / 2))

        v512 = nc.gpsimd.alloc_register("n512")
        nc.gpsimd.reg_mov(v512, 512)

        for g in range(G):
            # ---------------- gathers ----------------
            sidx = gpool.tile([128, 32], I16, tag="sidx")
            nc.sync.dma_start(sidx[:], P["src_w"][g])
            didx = gpool.tile([128, 32], I16, tag="didx")
            nc.sync.dma_start(didx[:], P["dst_w"][g])
            srf = gpool.tile([128, 4], F32, tag="srf")
            nc.sync.dma_start(srf[:], P["srcrel"][g])

            xs = gpool.tile([128, 512], BF16, tag="xs")
            nc.gpsimd.dma_gather(
                out_ap=xs[:].rearrange("p (c n) -> p c n", c=1),
                in_ap=P["xs_tab"][:], idxs_ap=sidx[:],
                num_idxs=512, num_idxs_reg=v512, elem_size=128, transpose=True)
            xd = gpool.tile([128, 512], BF16, tag="xd")
            nc.gpsimd.dma_gather(
                out_ap=xd[:].rearrange("p (c n) -> p c n", c=1),
                in_ap=P["xd_tab"][:], idxs_ap=didx[:],
                num_idxs=512, num_idxs_reg=v512, elem_size=128, transpose=True)
            pos_s = gpool.tile([128, 4, 64], F32, tag="pos_s")
            nc.gpsimd.dma_gather(
                out_ap=pos_s[:], in_ap=P["ps_tab"][:], idxs_ap=sidx[:],
                num_idxs=512, num_idxs_reg=v512, elem_size=64, transpose=False)
            pos_d = gpool.tile([128, 4, 64], F32, tag="pos_d")
            nc.gpsimd.dma_gather(
                out_ap=pos_d[:], in_ap=P["pd_tab"][:], idxs_ap=didx[:],
                num_idxs=512, num_idxs_reg=v512, elem_size=64, transpose=False)

            # ------------- distance path (chunk [128, 4, *]) -------------
            d = wpool.tile([128, 4, 3], F32, tag="d")
            nc.vector.tensor_tensor(out=d[:], in0=pos_s[:, :, 0:3],
                                    in1=pos_d[:, :, 0:3], op=ALU.subtract)
            sq = wpool.tile([128, 4, 3], F32, tag="sq")
            nc.vector.tensor_tensor(out=sq[:], in0=d[:], in1=d[:], op=ALU.mult)
            s2 = wpool.tile([128, 4, 1], F32, tag="s2")
            nc.vector.tensor_reduce(out=s2[:], in_=sq[:], axis=AX.X, op=ALU.add)
            wt = wpool.tile([128, 4, 1], F32, tag="wt")
            w_ap = wt[:]
            nc.scalar.activation(w_ap, s2[:], AF.Sqrt)
            invw = wpool.tile([128, 4, 1], F32, tag="invw")
            nc.vector.reciprocal(invw[:], w_ap)
            iwt = wpool.tile([128, 4, 1], F32, tag="iwt")
            nc.vector.tensor_tensor(out=iwt[:], in0=w_ap, in1=invw[:], op=ALU.mult)
            nc.vector.tensor_scalar(iwt[:], iwt[:], 2.0, -1.0, ALU.subtract,
                                    ALU.mult)
            nc.vector.tensor_tensor(out=invw[:], in0=invw[:], in1=iwt[:],
                                    op=ALU.mult)
            ev = wpool.tile([128, 4, 3], F32, tag="ev")
            nc.vector.tensor_tensor(out=ev[:], in0=d[:],
                                    in1=invw[:].to_broadcast([128, 4, 3]),
                                    op=ALU.mult)
            ew3 = wpool.tile([128, 4, 1], F32, tag="ew3")
            nc.vector.tensor_tensor(out=ew3[:], in0=w_ap, in1=s2[:], op=ALU.mult)
            # cutoff C = 0.5*(cos(pi*w/5)+1) * (w < 5)
            wcl = wpool.tile([128, 4, 1], F32, tag="wcl")
            nc.vector.tensor_scalar(wcl[:], w_ap, 5.0, None, ALU.min)
            sinv = wpool.tile([128, 4, 1], F32, tag="sinv")
            nc.scalar.activation(sinv[:], wcl[:], AF.Sin,
                                 bias=halfpi[:, 0:1], scale=float(-np.pi / OUTER))
            mlt = wpool.tile([128, 4, 1], F32, tag="mlt")
            nc.vector.tensor_scalar(mlt[:], w_ap, 5.0, None, ALU.is_lt)
            c0 = wpool.tile([128, 4, 1], F32, tag="c0")
            nc.vector.tensor_scalar(c0[:], sinv[:], 0.5, 0.5, ALU.mult, ALU.add)
            cC = wpool.tile([128, 4, 1], F32, tag="cC")
            nc.vector.tensor_tensor(out=cC[:], in0=c0[:], in1=mlt[:], op=ALU.mult)

            # bundle (bf16): [C, ew3, ews, w, 0...]
            bundle = wpool.tile([128, 4, 16], BF16, tag="bundle")
            nc.gpsimd.memset(bundle[:], 0.0)
            nc.vector.tensor_copy(bundle[:, :, 0:1], cC[:])
            nc.vector.tensor_copy(bundle[:, :, 1:2], ew3[:])
            nc.scalar.activation(bundle[:, :, 2:3], w_ap, AF.Sqrt)   # ews
            nc.vector.tensor_copy(bundle[:, :, 3:4], w_ap)

            # RBF: attr = exp(coeff*(off - x)^2), x in {w, w^3}; then * C
            q = wpool.tile([128, 4, 64], F32, tag="q")
            nc.vector.tensor_tensor(out=q[:, :, 0:32], in0=offm_sb[:],
                                    in1=w_ap.to_broadcast([128, 4, 32]),
                                    op=ALU.subtract)
            nc.vector.tensor_tensor(out=q[:, :, 32:64], in0=offm_sb[:],
                                    in1=ew3[:].to_broadcast([128, 4, 32]),
                                    op=ALU.subtract)
            q2 = wpool.tile([128, 4, 64], F32, tag="q2")
            nc.scalar.activation(q2[:], q[:], AF.Square)
            attr_f = wpool.tile([128, 4, 64], F32, tag="attr_f")
            nc.scalar.activation(attr_f[:], q2[:], AF.Exp, scale=COEFF)
            attr_b = wpool.tile([128, 4, 64], BF16, tag="attr_b")
            nc.vector.tensor_tensor(out=attr_b[:], in0=attr_f[:],
                                    in1=cC[:].to_broadcast([128, 4, 64]),
                                    op=ALU.mult)

            # ------------- gates (e-partition chain) -------------
            tdiff = wpool.tile([128, 4, 16], F32, tag="tdiff")
            nc.vector.tensor_tensor(out=tdiff[:], in0=tmat_sb[:],
                                    in1=w_ap.to_broadcast([128, 4, 16]),
                                    op=ALU.subtract)
            ntd = wpool.tile([128, 4, 16], F32, tag="ntd")
            nc.vector.tensor_scalar(ntd[:], tdiff[:], -1.0, None, ALU.mult)
            y = wpool.tile([128, 4, 16], F32, tag="y")
            nc.vector.tensor_tensor(out=y[:], in0=tdiff[:], in1=ntd[:],
                                    op=ALU.max)
            nc.vector.tensor_scalar(y[:], y[:], 1e-8, None, ALU.max)
            inv0 = wpool.tile([128, 4, 16], F32, tag="inv0")
            nc.vector.reciprocal(inv0[:], y[:])
            nt1 = wpool.tile([128, 4, 16], F32, tag="nt1")
            nc.vector.tensor_tensor(out=nt1[:], in0=y[:], in1=inv0[:], op=ALU.mult)
            nc.vector.tensor_scalar(nt1[:], nt1[:], 2.0, -1.0, ALU.subtract,
                                    ALU.mult)
            inv2 = wpool.tile([128, 4, 16], F32, tag="inv2")
            nc.vector.tensor_tensor(out=inv2[:], in0=inv0[:], in1=nt1[:],
                                    op=ALU.mult)
            mne = wpool.tile([128, 4, 1], F32, tag="mne")
            nc.vector.tensor_reduce(out=mne[:], in_=inv2[:, :, 0:10], axis=AX.X,
                                    op=ALU.max, negate=True)
            hm = wpool.tile([128, 4, 10], F32, tag="hm")
            nc.vector.tensor_tensor(out=hm[:], in0=inv2[:, :, 0:10],
                                    in1=mne[:].to_broadcast([128, 4, 10]),
                                    op=ALU.add)
            gexp = wpool.tile([128, 4, 16], F32, tag="gexp")
            nc.gpsimd.memset(gexp[:], 0.0)
            nc.scalar.activation(gexp[:, :, 0:10], hm[:], AF.Exp)
            s_e = wpool.tile([128, 4, 1], F32, tag="s_e")
            nc.vector.tensor_reduce(out=s_e[:], in_=gexp[:, :, 0:10], axis=AX.X,
                                    op=ALU.add)
            rs_e = wpool.tile([128, 4, 1], F32, tag="rs_e")
            nc.vector.reciprocal(rs_e[:], s_e[:])
            rt1 = wpool.tile([128, 4, 1], F32, tag="rt1")
            nc.vector.tensor_tensor(out=rt1[:], in0=s_e[:], in1=rs_e[:],
                                    op=ALU.mult)
            nc.vector.tensor_scalar(rt1[:], rt1[:], 2.0, -1.0, ALU.subtract,
                                    ALU.mult)
            nc.vector.tensor_tensor(out=rs_e[:], in0=rs_e[:], in1=rt1[:],
                                    op=ALU.mult)
            # normalized gates (bf16) + ev-scaled copies for the bias scatter
            gb = wpool.tile([128, 4, 64], BF16, tag="gb")
            nc.gpsimd.memset(gb[:], 0.0)
            nc.vector.tensor_tensor(out=gb[:, :, 0:10], in0=gexp[:, :, 0:10],
                                    in1=rs_e[:].to_broadcast([128, 4, 10]),
                                    op=ALU.mult)
            for i in range(3):
                nc.vector.tensor_tensor(
                    out=gb[:, :, 16 * (i + 1):16 * (i + 1) + 10],
                    in0=gb[:, :, 0:10],
                    in1=ev[:, :, i:i + 1].to_broadcast([128, 4, 10]),
                    op=ALU.mult)
            # rs*ev slot scales (f32)
            rsev = wpool.tile([128, 4, 3], F32, tag="rsev")
            nc.vector.tensor_tensor(out=rsev[:], in0=ev[:],
                                    in1=rs_e[:].to_broadcast([128, 4, 3]),
                                    op=ALU.mult)

            # ------------- per-subtile transposes -------------
            attrT = bpool.tile([64, 512], BF16, tag="attrT")
            bunT = bpool.tile([16, 512], BF16, tag="bunT")
            gexpT = bpool.tile([16, 512], BF16, tag="gexpT")
            for s in range(4):
                pt1 = ps_t.tile([64, 128], BF16, tag="pst", space="PSUM")
                nc.tensor.transpose(out=pt1[:], in_=attr_b[:, s], identity=eye_b[:])
                nc.scalar.copy(attrT[:, 128 * s:128 * (s + 1)], pt1[:])
                pt2 = ps_t.tile([16, 128], BF16, tag="pst", space="PSUM")
                nc.tensor.transpose(out=pt2[:], in_=bundle[:, s], identity=eye_b[:])
                nc.vector.tensor_copy(bunT[:, 128 * s:128 * (s + 1)], pt2[:])
                pt3 = ps_t.tile([16, 128], F32, tag="pst", space="PSUM")
                nc.tensor.transpose(out=pt3[:], in_=gexp[:, s], identity=eye_f[:])
                nc.vector.tensor_copy(gexpT[:, 128 * s:128 * (s + 1)], pt3[:])

            gexF = bpool.tile([1, 10, 512], BF16, tag="gexF")
            nc.sync.dma_start(gexF[:], gexpT[0:10, :])

            # ------------- dp + gamma matmuls -------------
            dp_sb = bpool.tile([128, 2, 512], BF16, tag="dp_sb")
            for m in range(2):
                pdp = ps_mm.tile([128, 512], F32, tag="psmm", space="PSUM")
                nc.tensor.matmul(pdp[:], lhsT=wdp_sb[:, m], rhs=attrT[:],
                                 start=True, stop=True)
                nc.scalar.copy(dp_sb[:, m], pdp[:])
            gam_sb = bpool.tile([128, 2, 512], BF16, tag="gam_sb")
            for m in range(2):
                pg = ps_mm.tile([128, 512], F32, tag="psmm", space="PSUM")
                nc.tensor.matmul(pg[:], lhsT=wc1_sb[:, 0, m], rhs=dp_sb[:, 0],
                                 start=True, stop=False)
                nc.tensor.matmul(pg[:], lhsT=wc1_sb[:, 1, m], rhs=dp_sb[:, 1],
                                 start=False, stop=False)
                nc.tensor.matmul(pg[:], lhsT=w1c_sb[:, m], rhs=xs[:],
                                 start=False, stop=False)
                nc.tensor.matmul(pg[:], lhsT=w2c_sb[:, m], rhs=xd[:],
                                 start=False, stop=False)
                nc.tensor.matmul(pg[:], lhsT=r1_sb[:, m], rhs=bunT[:],
                                 start=False, stop=True)
                nc.scalar.activation(gam_sb[:, m], pg[:], AF.Identity,
                                     bias=bias_sb[:, m:m + 1])

            # ------------- experts with gate prescale -------------
            pel = ps_mm.tile([128, 512], F32, tag="psmm", space="PSUM")
            for g10 in range(NUM_GATES):
                gbc = scpool.tile([128, 512], BF16, tag="gbc")
                nc.gpsimd.partition_broadcast(gbc[:], gexF[:, g10])
                gs0 = scpool.tile([128, 512], BF16, tag="gs0")
                nc.vector.tensor_tensor(out=gs0[:], in0=gam_sb[:, 0], in1=gbc[:],
                                        op=ALU.mult)
                gs1 = scpool.tile([128, 512], BF16, tag="gs1")
                nc.vector.tensor_tensor(out=gs1[:], in0=gam_sb[:, 1], in1=gbc[:],
                                        op=ALU.mult)
                nc.tensor.matmul(pel[:], lhsT=wexp_sb[:, g10, 0], rhs=gs0[:],
                                 start=(g10 == 0), stop=False)
                nc.tensor.matmul(pel[:], lhsT=wexp_sb[:, g10, 1], rhs=gs1[:],
                                 start=False, stop=(g10 == NUM_GATES - 1))
            el_b = bpool.tile([128, 512], BF16, tag="el_b")
            nc.scalar.copy(el_b[:], pel[:])

            # ------------- edge_level transpose + slots + scatter -------------
            psc = ps_sc.tile([128, 4, 128], F32, tag="pssc", space="PSUM")
            psc5 = ps_g5.tile([64, 128], F32, tag="psg5", space="PSUM")
            elTs, ohs = [], []
            for s in range(4):
                ptl = ps_t.tile([128, 128], BF16, tag="pst", space="PSUM")
                nc.tensor.transpose(out=ptl[:], in_=el_b[:, 128 * s:128 * (s + 1)],
                                    identity=eye_b[:])
                elT = scpool.tile([128, 4, 128], BF16, tag="elT", name=f"elT{s}")
                nc.scalar.activation(elT[:, 0], ptl[:], AF.Copy,
                                     scale=rs_e[:, s])
                for i in range(3):
                    nc.scalar.activation(elT[:, i + 1], ptl[:], AF.Copy,
                                         scale=rsev[:, s, i:i + 1])
                oh = scpool.tile([128, 128], BF16, tag="oh", name=f"oh{s}")
                nc.vector.tensor_tensor(out=oh[:],
                                        in0=srf[:, s:s + 1].to_broadcast([128, 128]),
                                        in1=iota_sb[:], op=ALU.is_equal)
                elTs.append(elT)
                ohs.append(oh)
                nc.tensor.matmul(psc5[:], lhsT=gb[:, s], rhs=oh[:],
                                 start=(s == 0), stop=(s == 3))
            for t in range(4):
                for s in range(4):
                    nc.tensor.matmul(psc[:, t], lhsT=elTs[s][:, t], rhs=ohs[s],
                                     start=(s == 0), stop=(s == 3))

            # ------------- window flush to DRAM -------------
            fl = bpool.tile([128, 4, 128], F32, tag="fl")
            nc.scalar.copy(fl[:], psc[:])
            nc.sync.dma_start(P["out_w"][g],
                              fl[:].rearrange("p t a -> p (t a)"))
            fl5 = bpool.tile([64, 128], F32, tag="fl5")
            nc.vector.tensor_copy(fl5[:], psc5[:])
            nc.sync.dma_start(P["outg_w"][g], fl5[:])


    nc.finalize()
    return nc, P


# ====================== host-side preparation ======================

def prep_core(src, dst, xz_bf16, pos, cfg):
    """Build the per-core input map. src/dst: this core's edges (global ids,
    sorted by src). xz_bf16: [N_ATOMS,128] bf16 node features. pos: [N,3]."""
    OWN_PAD, ACC_N, DU_PAD, G = (cfg["OWN_PAD"], cfg["ACC_N"], cfg["DU_PAD"],
                                 cfg["G"])
    EPC = 512 * G
    core = int(src[0]) // OWN if len(src) else 0
    base = core * OWN
    n = len(src)
    assert n <= EPC, (n, EPC)

    src_local = (src - base).astype(np.int64)
    du = np.unique(dst)
    assert len(du) <= DU_PAD - 1, len(du)
    dstc = np.searchsorted(du, dst).astype(np.int64)

    sl_p = np.full(EPC, OWN_PAD - 1, np.int64)
    sl_p[:n] = src_local
    dc_p = np.full(EPC, DU_PAD - 1, np.int64)
    dc_p[:n] = dstc

    A0 = np.zeros(G, np.int32)
    srcrel = np.full(EPC, -1000.0, np.float32)
    for g in range(G):
        lo = g * 512
        hi = min(lo + 512, n)
        if lo < n:
            a0 = int(sl_p[lo])
            A0[g] = a0
            srcrel[lo:hi] = (sl_p[lo:hi] - a0).astype(np.float32)
            assert sl_p[hi - 1] - a0 <= 127, "window overflow"

    def wrap(idx):
        w = idx.astype(np.int16).reshape(G, 32, 16)
        w = np.transpose(w, (0, 2, 1))          # [G, 16, 32]
        return np.ascontiguousarray(np.tile(w, (1, 8, 1)))

    src_w = wrap(sl_p)
    dst_w = wrap(dc_p)
    srcrel_T = np.ascontiguousarray(
        np.transpose(srcrel.reshape(G, 4, 128), (0, 2, 1)))

    xs_tab = np.zeros((OWN_PAD, 128), BF16NP)
    xs_tab[:OWN] = xz_bf16[base:base + OWN]
    xd_tab = np.zeros((DU_PAD, 128), BF16NP)
    xd_tab[:len(du)] = xz_bf16[du]

    ps_tab = np.zeros((OWN_PAD, 64), np.float32)
    ps_tab[:OWN, 0:3] = pos[base:base + OWN]
    pd_tab = np.ones((DU_PAD, 64), np.float32)
    pd_tab[:len(du), 0:3] = pos[du]
    pd_tab[len(du):, 0:3] = 1.0  # pad rows: finite, != src pad pos

    return dict(xs_tab=xs_tab, xd_tab=xd_tab, ps_tab=ps_tab, pd_tab=pd_tab,
                src_w=src_w, dst_w=dst_w, srcrel=srcrel_T,
                a0s=A0.reshape(1, G)), du, n


def prep_weights(inputs, cfg):
    """Fused weight tensors shared by all cores (host math in float64)."""
    W_gamma = np.asarray(inputs["W_gamma"], np.float64)
    Wg1 = W_gamma[:, 0:128]
    Wg2 = W_gamma[:, 128:384]
    Wg3 = W_gamma[:, 384:512]
    W_ai = np.asarray(inputs["W_ai"], np.float64)
    W_aj = np.asarray(inputs["W_aj"], np.float64)
    W_dij = np.asarray(inputs["W_dij"], np.float64)
    W_dp = np.asarray(inputs["W_dp"], np.float64)
    b_dp = np.asarray(inputs["b_dp"], np.float64)
    b_ai = np.asarray(inputs["b_ai"], np.float64)
    b_aj = np.asarray(inputs["b_aj"], np.float64)
    b_dij = np.asarray(inputs["b_dij"], np.float64)
    b_gamma = np.asarray(inputs["b_gamma"], np.float64)
    W_exp = np.asarray(inputs["W_exp"], np.float64)   # [10, 128, 256]
    t_params = np.asarray(inputs["t_params"], np.float64)

    W1c = Wg1 @ W_ai                    # [256, 128]
    W2c = Wg2 @ W_aj                    # [256, 128]
    Wc = Wg3 @ W_dij                    # [256, 259]
    Wc1 = Wc[:, 0:256]
    vbc = Wc1 @ b_dp                    # [256]
    bias_total = b_gamma + Wg1 @ b_ai + Wg2 @ b_aj + Wg3 @ b_dij

    def b16(x):
        return np.ascontiguousarray(x.astype(np.float32)).astype(BF16NP)

    # wdpT [k=64 attr, mh, m]
    wdpT = b16(W_dp.T.reshape(64, 2, 128))
    # wc1T [k_lo, kh, mh, m] from Wc1.T [k=256, m=256]
    wc1T = b16(np.transpose(Wc1.T.reshape(2, 128, 2, 128), (1, 0, 2, 3)))
    w1cT = b16(W1c.T.reshape(128, 2, 128))
    w2cT = b16(W2c.T.reshape(128, 2, 128))
    r1 = np.zeros((16, 2, 128), BF16NP)
    r1[0] = b16(vbc.reshape(2, 128))
    r1[1] = b16(Wc[:, 256].reshape(2, 128))
    r1[2] = b16(Wc[:, 257].reshape(2, 128))
    r1[3] = b16(Wc[:, 258].reshape(2, 128))
    # wexpT [k_lo, g, kh, m] from W_exp[g].T [k=256, m=128]
    wexpT = b16(np.transpose(
        np.transpose(W_exp, (0, 2, 1)).reshape(10, 2, 128, 128), (2, 0, 1, 3)))

    t_pad = np.full(16, 1e9, np.float32)
    t_pad[:NUM_GATES] = t_params.astype(np.float32)
    tmat = np.ascontiguousarray(
        np.broadcast_to(t_pad, (128, 4, 16)).astype(np.float32))
    offm = np.ascontiguousarray(
        np.broadcast_to(_OFF.astype(np.float32), (128, 4, 32)))
    iota = np.ascontiguousarray(
        np.broadcast_to(np.arange(128, dtype=np.float32), (128, 128)))
    bias_g = np.ascontiguousarray(
        bias_total.astype(np.float32).reshape(2, 128).T)   # [128, 2]

    return dict(wdpT=wdpT, wc1T=wc1T, w1cT=w1cT, w2cT=w2cT, r1=r1,
                wexpT=wexpT, tmat=tmat, offmat=offm, iota=iota, bias_g=bias_g)


_PROG_CACHE = {}
LAST_EXEC_NS = None
LAST_RESULTS = None


def kernel(**inputs):
    cfg = CFG
    z = np.asarray(inputs["z"])
    pos = np.asarray(inputs["pos"], np.float32)
    batch = np.asarray(inputs["batch"])
    edge_index = np.asarray(inputs["edge_index"])
    emb_table = np.asarray(inputs["emb_table"], np.float32)
    b_exp = np.asarray(inputs["b_exp"], np.float32)     # [10, 128]

    src = edge_index[0].astype(np.int64)
    dst = edge_index[1].astype(np.int64)
    xz = emb_table[z].astype(BF16NP)                    # [N, 128] bf16

    wmaps = prep_weights(inputs, cfg)

    in_maps = []
    metas = []
    core_id = src // OWN
    for c in range(NCORES):
        m = core_id == c
        es, ed = src[m], dst[m]
        order = np.argsort(es, kind="stable")
        es, ed = es[order], ed[order]
        core_in, du, n = prep_core(es, ed, xz, pos, cfg)
        core_in.update(wmaps)
        in_maps.append(core_in)
        metas.append(du)

    key = "full"
    if key not in _PROG_CACHE:
        _PROG_CACHE[key] = build_program(cfg)
    nc, _ = _PROG_CACHE[key]

    import os
    from concourse.bass_utils import run_bass_kernel_spmd
    trace = bool(int(os.environ.get("KERNEL_TRACE", "0")))
    try:
        res = run_bass_kernel_spmd(nc, in_maps, list(range(NCORES)), trace=trace)
    except ModuleNotFoundError:
        res = run_bass_kernel_spmd(nc, in_maps, list(range(NCORES)))
    global LAST_EXEC_NS, LAST_RESULTS
    LAST_EXEC_NS = res.exec_time_ns
    LAST_RESULTS = res

    atom_x = np.zeros((N_ATOMS, 128), np.float32)
    vec = np.zeros((N_ATOMS, 3, 128), np.float32)
    G = cfg["G"]
    ACC_N = cfg["ACC_N"]
    for c in range(NCORES):
        out_w = res.results[c]["out_w"]        # [G, 128, 512]
        outg_w = res.results[c]["outg_w"]      # [G, 64, 128]
        A0 = np.asarray(in_maps[c]["a0s"]).reshape(G)
        out_t = np.zeros((512, ACC_N), np.float64)
        accg = np.zeros((64, ACC_N), np.float64)
        for g in range(G):
            a0 = int(A0[g])
            out_t[:, a0:a0 + 128] += out_w[g].reshape(128, 4, 128).transpose(
                1, 0, 2).reshape(512, 128)
            accg[:, a0:a0 + 128] += outg_w[g]
        full = out_t[:, :OWN].T                # [OWN, 512]
        gsum = accg[0:10, :OWN].T              # [OWN, 10]
        gev = accg[16:64, :OWN].reshape(3, 16, OWN)[:, 0:10]  # [3, 10, OWN]
        ax = full[:, 0:128] + gsum @ b_exp
        vv = full[:, 128:512].reshape(OWN, 3, 128)
        vv = vv + np.einsum("ign,gc->nic", gev, b_exp)
        atom_x[c * OWN:(c + 1) * OWN] = ax.astype(np.float32)
        vec[c * OWN:(c + 1) * OWN] = vv.astype(np.float32)

    return (atom_x, vec, np.asarray(inputs["z"]), np.asarray(inputs["pos"]),
            np.asarray(inputs["batch"]))
